# revision 1
# baseline (speedup 1.0000x reference)
"""CRF forward (logsumexp over paths) loss kernel for Trainium2, 8 NeuronCores.

Math
----
reference:  fv0 = alpha_0^T + emits[0]                       [B, K]
            fv_t[b,j] = logsumexp_i(fv_{t-1}[b,i] + trans[i,j]) + emit_t[b,j]
            alpha_z = sum_b logsumexp_k( fv_{tau_b}[b,:] )   (tau = one-hot mask step)

We run the recurrence in exp space.  With ETs[i,j] = exp(trans[i,j] - delta)
and e_t[j,b] = exp(emit_t[b,j]) (transposed), the state w_t[j,b] =
exp(fv_t[j,b] - delta*t - C[b]) obeys

    w_t = (ETs^T w_{t-1}) * e_t        (one matmul + one elementwise mul)

C[b] tracks periodic renormalizations (every W=8 steps we divide by a recent
column sum and add its log to C).  The transition weight matrix is augmented
with a 65th column of ones so each matmul also emits colsum(w_{t-1}) in PSUM
row 64; the elementwise multiply covers 65 rows (the transposed-emission tile
has a preset row of ones), so every step's column sum is captured into a
history buffer for free.  The one-hot time mask turns the final
"select alpha at tau_b" into a linear masked sum over that colsum history:

    result[b] = log( sum_s mask[s-1,b] * colsum_{s-1}[b] ) + C_win(s)[b] + delta*tau_b

Sharding: batch B=512 split across 8 cores (64 per core); transitions/alpha_0
replicated; final alpha_z = host sum of the 8 per-core [1,64] row outputs.
"""

import os
import sys

for _p in ("/opt/trn_rl_repo", "/root/.axon_site/_ro/trn_rl_repo"):
    if os.path.isdir(_p) and _p not in sys.path:
        sys.path.insert(0, _p)

from contextlib import ExitStack

import numpy as np

import concourse.bass as bass
import concourse.mybir as mybir
import concourse.tile as tile
from concourse.bass_utils import run_bass_kernel_spmd
from concourse.masks import make_identity

# The walrus build in this container rejects instructions carrying more than
# one sync-wait command ("Too many sync wait commands" in setupSyncWait).
# Tile freely emits multi-wait instructions, so split the extras onto
# preceding same-engine no-ops at commit time (engine queues execute
# in-order, so the semantics are identical).
_ORIG_COMMIT = tile.TileContext._commit_instruction


def _single_wait_commit(self, inst, lazy_reg_writes=True):
    si = getattr(inst, "sync_info", None)
    if (
        si is not None
        and si.on_wait
        and len(si.on_wait) > 1
        and inst.engine != mybir.EngineType.Unassigned
    ):
        waits = list(si.on_wait)
        eng = self.nc.engines[inst.engine]
        for w in waits[:-1]:
            n = eng.nop(nofuse=True)
            n.ins.sync_info = mybir.SyncInfo(on_wait=[w], on_update=[])
        inst.sync_info = mybir.SyncInfo(
            on_wait=[waits[-1]], on_update=list(si.on_update or [])
        )
    _ORIG_COMMIT(self, inst, lazy_reg_writes)


tile.TileContext._commit_instruction = _single_wait_commit

T, B, K = 512, 512, 64
NCORES = 8
BSH = B // NCORES          # 64 batch elements per core
W = 8                      # slots per window (renorm/capture period)
NWINCHAIN = T // W         # 64 windows of chain steps (slots 0..511)
NWIN = NWINCHAIN + 1       # 65: slot 512 (colsum of t=511) lands in window 64
DELTA = 5.0                # static per-step log-space offset folded into ETs
ETRBUF = 24                # transposed-emission ring slots
F32 = mybir.dt.float32
BF16 = mybir.dt.bfloat16
U8 = mybir.dt.uint8
I32 = mybir.dt.int32
MULT = mybir.AluOpType.mult
ADD = mybir.AluOpType.add
AX = mybir.AxisListType.X
AF = mybir.ActivationFunctionType


def _build_crf_nc() -> bass.Bass:
    nc = bass.Bass(trn_type="TRN2", target_bir_lowering=False, debug=False)

    emits_d = nc.dram_tensor("emits", [T, BSH, K], F32, kind="ExternalInput").ap()
    mask_d = nc.dram_tensor("maskb", [T, BSH], U8, kind="ExternalInput").ap()
    trans_d = nc.dram_tensor("transitions", [K, K], F32, kind="ExternalInput").ap()
    alpha0_d = nc.dram_tensor("alpha_0", [K, 1], F32, kind="ExternalInput").ap()
    out_d = nc.dram_tensor("out_row", [1, BSH], F32, kind="ExternalOutput").ap()

    with tile.TileContext(nc) as tc:
        with ExitStack() as ctx:
            _crf_body(ctx, tc, emits_d, mask_d, trans_d, alpha0_d, out_d)
    _split_remaining_multiwaits(nc)
    return nc


def _split_remaining_multiwaits(nc):
    """Split multi-wait instructions added outside the commit path (e.g. the
    end-of-kernel drain/barrier) onto preceding same-engine no-ops."""
    for blk in nc.m.functions[0].blocks:
        il = blk.instructions
        idx = 0
        while idx < len(il):
            inst = il[idx]
            si = inst.sync_info
            if si is not None and si.on_wait and len(si.on_wait) > 1:
                waits = list(si.on_wait)
                for j, w in enumerate(waits[:-1]):
                    n = mybir.InstNoOp(
                        name=f"I-swx-{inst.name}-{j}", ins=[], outs=[]
                    )
                    n.engine = inst.engine
                    n.sync_info = mybir.SyncInfo(on_wait=[w], on_update=[])
                    nc.register_instruction(n, overwrite=True)
                    il.insert(idx, n)
                    idx += 1
                inst.sync_info = mybir.SyncInfo(
                    on_wait=[waits[-1]], on_update=list(si.on_update or [])
                )
            idx += 1


def _crf_body(ctx, tc, emits_d, mask_d, trans_d, alpha0_d, out_d):
    nc = tc.nc

    # ---- long-lived SBUF state ----
    ets = nc.alloc_sbuf_tensor("ets", [K, K + 1], BF16).ap()        # exp(trans-d)|1
    expal = nc.alloc_sbuf_tensor("expal", [K, 1], F32).ap()        # exp(alpha_0)
    # chain state ring: 2 window buffers x W slots x BSH cols, 65 rows
    # (row 64 of slot s = colsum of w_{s-1})
    w_all = nc.alloc_sbuf_tensor("w_all", [K + 1, 2 * W * BSH], BF16).ap()
    wrn = nc.alloc_sbuf_tensor("wrn", [K, 2 * BSH], BF16).ap()      # renormed state
    c_rows = nc.alloc_sbuf_tensor("c_rows", [1, 2 * BSH], F32).ap()  # C ping-pong
    en_ring = nc.alloc_sbuf_tensor("en_ring", [BSH, 2 * W * 2 * K], BF16).ap()
    ident = nc.alloc_sbuf_tensor("ident", [BSH, BSH], BF16).ap()
    etr_sb = nc.alloc_sbuf_tensor("etr_sb", [K + 1, 2 * W * BSH], BF16).ap()
    csum = nc.alloc_sbuf_tensor("csum", [NWIN, W * BSH], BF16).ap()  # colsum history
    c_hist = nc.alloc_sbuf_tensor("c_hist", [NWIN, BSH], F32).ap()  # log-norm per win
    maskw = nc.alloc_sbuf_tensor("maskw", [NWIN, W * BSH], F32).ap()
    mk_u8 = nc.alloc_sbuf_tensor("mk_u8", [NWIN, W * BSH], U8).ap()
    iota_i = nc.alloc_sbuf_tensor("iota_i", [NWIN, W * BSH], I32).ap()
    iotaw = nc.alloc_sbuf_tensor("iotaw", [NWIN, W * BSH], F32).ap()
    ones_c = nc.alloc_sbuf_tensor("ones_c", [NWIN, 1], F32).ap()   # partition-reduce
    ones_r = nc.alloc_sbuf_tensor("ones_r", [1, K], F32).ap()      # row broadcast
    cst = nc.alloc_sbuf_tensor("cst", [K, 2], F32).ap()            # bias constants

    # ---- pools ----
    em_pool = ctx.enter_context(tc.tile_pool(name="em", bufs=3))
    etp_pool = ctx.enter_context(tc.tile_pool(name="etp", bufs=3, space="PSUM"))
    ps_pool = ctx.enter_context(tc.tile_pool(name="ps", bufs=3, space="PSUM"))
    psb_pool = ctx.enter_context(tc.tile_pool(name="psb", bufs=2, space="PSUM"))
    row_pool = ctx.enter_context(tc.tile_pool(name="rows", bufs=6))
    fin_pool = ctx.enter_context(tc.tile_pool(name="fin", bufs=1))

    # ---- one-time setup ----
    nc.vector.memset(w_all[K : K + 1, 0:BSH], 0.0)  # slot 0 has no colsum
    # emission staging: per step a [BSH, 128] block, col 64 = 1.0 (becomes the
    # ones row of the transposed tile -> colsum row of the state)
    nc.gpsimd.memset(en_ring[:, :], 0.0)
    nc.vector.memset(
        en_ring.rearrange("b (s c) -> b s c", c=2 * K)[:, :, K : K + 1], 1.0
    )
    nc.gpsimd.memset(csum[:, :], 0.0)
    nc.gpsimd.memset(c_hist[:, :], 0.0)
    nc.gpsimd.memset(c_rows[:, :], 0.0)
    nc.gpsimd.memset(mk_u8[:, :], 0)
    nc.gpsimd.memset(ones_c[:, :], 1.0)
    nc.gpsimd.memset(ones_r[:, :], 1.0)
    nc.gpsimd.memset(cst[:, 0:1], 0.0)
    nc.gpsimd.memset(cst[:, 1:2], -DELTA)
    make_identity(nc, ident)

    tr_t = fin_pool.tile([K, K], F32)
    nc.sync.dma_start(tr_t[:], trans_d)
    nc.scalar.activation(ets[:, 0:K], tr_t[:], AF.Exp, bias=cst[0:K, 1:2])
    nc.vector.memset(ets[:, K : K + 1], 1.0)

    a0_t = fin_pool.tile([K, 1], F32)
    nc.sync.dma_start(a0_t[:], alpha0_d)
    nc.scalar.activation(expal, a0_t[:], AF.Exp, bias=cst[0:K, 0:1])

    # mask (one-hot over t, per b) -> slot layout: slot s <-> t = s-1.
    # maskw[win, tw*BSH + b] = mask[win*W + tw - 1, b]
    nc.sync.dma_start(
        mk_u8[0:1, BSH : W * BSH],
        mask_d[0 : W - 1].rearrange("(o t) b -> o (t b)", o=1),
    )
    nc.sync.dma_start(
        mk_u8[1:NWINCHAIN, :],
        mask_d[W - 1 : T - 1].rearrange("(w t) b -> w t b", t=W),
    )
    nc.sync.dma_start(mk_u8[NWINCHAIN : NWIN, 0:BSH], mask_d[T - 1 : T])
    nc.vector.tensor_copy(maskw[:, :], mk_u8[:, :])
    # t value at each slot position (t = win*W + tw - 1)
    nc.gpsimd.iota(iota_i[:, :], pattern=[[1, W], [0, BSH]], base=-1,
                   channel_multiplier=W)
    nc.vector.tensor_copy(iotaw[:, :], iota_i[:, :])

    def w_off(t):  # column offset of chain slot t in w_all
        return ((t // W) % 2) * (W * BSH) + (t % W) * BSH

    def en_blk(t):
        par = (t // W) % 2
        v = en_ring.rearrange("b (s c) -> b s c", c=2 * K)
        return v[:, par * W + (t % W), :]

    def etr_sl(t):
        o = ((t // W) % 2) * (W * BSH) + (t % W) * BSH
        return etr_sb[:, o : o + BSH]

    # ---- emission streaming: load chunk of W steps, exp -> bf16, transpose ----
    def load_chunk(t0):
        em = em_pool.tile([BSH, W * K], F32)
        nc.sync.dma_start(
            em[:].rearrange("b (t k) -> b t k", t=W),
            emits_d[t0 : t0 + W].rearrange("t b k -> b t k"),
        )
        par = (t0 // W) % 2
        blk = en_ring.rearrange("b (s c) -> b s c", c=2 * K)[
            :, par * W : (par + 1) * W, :
        ]
        nc.scalar.activation(
            blk[:, :, 0:K],
            em[:].rearrange("b (t k) -> b t k", t=W),
            AF.Exp,
            bias=cst[0:BSH, 0:1],
        )

    # ---- chain ----
    # Per window of W steps: 8 PE transposes land in ONE PSUM tile (columns),
    # then one ACT copy moves the whole window to SBUF.  PE transposes for
    # window w+1 are interleaved one-per-chain-step so they hide in the PE
    # idle time of the serial chain.
    load_chunk(0)
    load_chunk(W)
    etp_cur = etp_pool.tile([K + 1, W * BSH], BF16, tag="etp")
    for tw in range(W):
        nc.tensor.transpose(
            etp_cur[:, tw * BSH : (tw + 1) * BSH], en_blk(tw)[:, 0 : K + 1], ident
        )
    nc.scalar.copy(etr_sb[:, 0 : W * BSH], etp_cur[:])
    # t = 0: w_0 = exp(alpha_0) * e_0
    nc.vector.tensor_scalar(
        w_all[0:K, 0:BSH], etr_sl(0)[0:K, :], expal, None, op0=MULT
    )
    etp_cur = etp_pool.tile([K + 1, W * BSH], BF16, tag="etp")  # window 1
    nc.tensor.transpose(etp_cur[:, 0:BSH], en_blk(W)[:, 0 : K + 1], ident)

    HB = BSH // 2  # sub-chain half width
    bc_cur = None
    for t in range(1, T):
        if t % W == 0 and t + 2 * W <= T:
            load_chunk(t + W)
        if t % W == 0 and t + W < T:
            etp_cur = etp_pool.tile([K + 1, W * BSH], BF16, tag="etp")
        s = t + W  # lookahead transpose for slot s
        if s < T:
            nc.tensor.transpose(
                etp_cur[:, (s % W) * BSH : (s % W + 1) * BSH],
                en_blk(s)[:, 0 : K + 1],
                ident,
            )
        if t % W == 4:
            # stale renorm prep (off the critical path): reciprocal +
            # broadcast of colsum_{t-2} (slot t-1's row 64); the scale is
            # applied at the window boundary and ln(s) booked into C.
            win = t // W
            woff = (win % 2) * (W * BSH)
            s_row = w_all[K : K + 1, woff + 3 * BSH : woff + 4 * BSH]
            ln_s = row_pool.tile([1, BSH], F32, tag="lns")
            nc.scalar.activation(ln_s[:], s_row, AF.Ln, bias=cst[0:1, 0:1])
            # 1/s as exp(-ln s) on the Scalar engine: keeps the reciprocal
            # off the busy DVE, and the booked ln_s matches the applied
            # factor by construction.
            rc = row_pool.tile([1, BSH], F32, tag="recip")
            nc.scalar.activation(rc[:], ln_s[:], AF.Exp, scale=-1.0,
                                 bias=cst[0:1, 0:1])
            bc_cur = psb_pool.tile([K, BSH], F32, tag="bc")
            nc.tensor.matmul(bc_cur[:], ones_r, rc[:], start=True, stop=True)
            pw, pr = ((win + 1) % 2) * BSH, (win % 2) * BSH
            nc.gpsimd.tensor_tensor(
                c_rows[:, pw : pw + BSH], c_rows[:, pr : pr + BSH], ln_s[:], op=ADD
            )
            nc.gpsimd.dma_start(c_hist[win + 1 : win + 2, :], c_rows[:, pw : pw + BSH])
        # rhs = previous state (renormed copy at window starts)
        if t % W == 0:
            wp = wrn[:, ((t // W - 1) % 2) * BSH :]
        else:
            o = w_off(t - 1)
            wp = w_all[0:K, o : o + BSH]
        o = w_off(t)
        # two independent 32-wide sub-chains hide each other's latency
        ps_a = ps_pool.tile([K + 1, HB], F32, tag="ps")
        nc.tensor.matmul(ps_a[:], ets[:, :], wp[0:K, 0:HB], start=True, stop=True)
        ps_b = ps_pool.tile([K + 1, HB], F32, tag="ps")
        nc.tensor.matmul(ps_b[:], ets[:, :], wp[0:K, HB:BSH], start=True, stop=True)
        nc.vector.tensor_tensor(
            w_all[0 : K + 1, o : o + HB], ps_a[:], etr_sl(t)[:, 0:HB], op=MULT
        )
        nc.vector.tensor_tensor(
            w_all[0 : K + 1, o + HB : o + BSH], ps_b[:], etr_sl(t)[:, HB:BSH],
            op=MULT,
        )
        if t % W == W - 1 and s < T:
            # window (t+W)//W fully transposed -> batch copy to SBUF
            wn = s // W
            nc.scalar.copy(
                etr_sb[:, (wn % 2) * W * BSH : ((wn % 2) + 1) * W * BSH],
                etp_cur[:],
            )

        if t % W == W - 1:
            win = t // W
            woff = (win % 2) * (W * BSH)
            # capture the window's colsum row into the history (DMA: compute
            # engines cannot write at arbitrary start partitions)
            nc.gpsimd.dma_start(
                csum[win : win + 1, :], w_all[K : K + 1, woff : woff + W * BSH]
            )
            # apply the pre-computed stale renorm scale
            nc.vector.tensor_tensor(
                wrn[:, (win % 2) * BSH :][:, 0:BSH],
                w_all[0:K, woff + (W - 1) * BSH : woff + W * BSH],
                bc_cur[:],
                op=MULT,
            )

    # slot 512: one extra matmul for colsum of w_{511}
    ps = ps_pool.tile([K + 1, BSH], F32, tag="ps")
    nc.tensor.matmul(
        ps[:], ets[:, :], wrn[:, (NWINCHAIN - 1) % 2 * BSH :][:, 0:BSH],
        start=True, stop=True,
    )
    nc.scalar.copy(csum[NWINCHAIN : NWIN, 0:BSH], ps[K : K + 1, :])

    # ---- final combine ----
    prodz = fin_pool.tile([NWIN, W * BSH], F32, tag="prodz")
    nc.vector.tensor_tensor(prodz[:], csum[:, :], maskw[:, :], op=MULT)
    redz = fin_pool.tile([NWIN, BSH], F32, tag="redz")
    nc.vector.tensor_reduce(
        redz[:], prodz[:].rearrange("p (t b) -> p b t", t=W), axis=AX, op=ADD
    )
    mwin = fin_pool.tile([NWIN, BSH], F32, tag="mwin")
    nc.vector.tensor_reduce(
        mwin[:], maskw[:, :].rearrange("p (t b) -> p b t", t=W), axis=AX, op=ADD
    )
    xc = fin_pool.tile([NWIN, BSH], F32, tag="xc")
    nc.vector.tensor_tensor(xc[:], mwin[:], c_hist[:, :], op=MULT)
    prodt = fin_pool.tile([NWIN, W * BSH], F32, tag="prodt")
    nc.vector.tensor_tensor(prodt[:], maskw[:, :], iotaw[:, :], op=MULT)
    redt = fin_pool.tile([NWIN, BSH], F32, tag="redt")
    nc.vector.tensor_reduce(
        redt[:], prodt[:].rearrange("p (t b) -> p b t", t=W), axis=AX, op=ADD
    )
    # xc += DELTA * redt
    nc.vector.scalar_tensor_tensor(xc[:], redt[:], DELTA, xc[:], op0=MULT, op1=ADD)

    accz = ps_pool.tile([1, BSH], F32, tag="ps")
    nc.tensor.matmul(accz[:], ones_c, redz[:], start=True, stop=True)
    accc = ps_pool.tile([1, BSH], F32, tag="ps")
    nc.tensor.matmul(accc[:], ones_c, xc[:], start=True, stop=True)
    lnz = row_pool.tile([1, BSH], F32, tag="lnz")
    nc.scalar.activation(lnz[:], accz[:], AF.Ln, bias=cst[0:1, 0:1])
    res = row_pool.tile([1, BSH], F32, tag="res")
    nc.vector.tensor_tensor(res[:], lnz[:], accc[:], op=ADD)
    nc.sync.dma_start(out_d, res[:])


_NC_CACHE = None


def _get_nc():
    global _NC_CACHE
    if _NC_CACHE is None:
        _NC_CACHE = _build_crf_nc()
    return _NC_CACHE


def _make_in_maps(np_inputs):
    emits = np.asarray(np_inputs["emits"], dtype=np.float32)
    mask_u8 = np.asarray(np_inputs["mask"]).astype(np.uint8)
    transitions = np.asarray(np_inputs["transitions"], dtype=np.float32)
    alpha_0 = np.asarray(np_inputs["alpha_0"], dtype=np.float32)
    in_maps = []
    for c in range(NCORES):
        sl = slice(c * BSH, (c + 1) * BSH)
        in_maps.append(
            {
                "emits": np.ascontiguousarray(emits[:, sl, :]),
                "maskb": np.ascontiguousarray(mask_u8[:, sl]),
                "transitions": transitions,
                "alpha_0": alpha_0,
            }
        )
    return in_maps


def kernel(emits, mask, transitions, alpha_0):
    nc = _get_nc()
    in_maps = _make_in_maps(
        {"emits": emits, "mask": mask, "transitions": transitions,
         "alpha_0": alpha_0}
    )
    res = run_bass_kernel_spmd(nc, in_maps, core_ids=list(range(NCORES)))
    total = np.float64(0.0)
    for r in res.results:
        total += np.asarray(r["out_row"], dtype=np.float64).sum()
    return np.float32(total)



# revision 9
# speedup vs baseline: 2.5942x; 2.5942x over previous
"""CRF forward (logsumexp over paths) loss kernel for Trainium2, 8 NeuronCores.

Chunk-parallel-in-time formulation
----------------------------------
reference:  fv0 = alpha_0^T + emits[0]                       [B, K]
            fv_t[b,j] = logsumexp_i(fv_{t-1}[b,i] + trans[i,j]) + emit_t[b,j]
            alpha_z = sum_b logsumexp_k( fv_{tau_b}[b,:] )   (tau = one-hot mask step)

In exp space the recurrence w_t = (ETs^T w_{t-1}) * e_t (ETs = exp(trans-DELTA))
is a product of strictly positive matrices, which contracts any two initial
states to the same *direction* at ~1e-1/step (Birkhoff).  So the time axis is
split into C=8 chunks of L=64 steps run CONCURRENTLY, each started from an
all-ones guess OV=16 steps early; after the burn-in the chunk states equal the
true states up to a per-column scalar.  That scalar is recovered exactly by
comparing log-colsums of adjacent chunks at the overlap boundary (a length-8
prefix sum done on the host from exported colsums).

The C concurrent chunks fuse into WIDE instructions: per round one
[64->65, 512] matmul (stationary never changes -> no LDWEIGHTS churn) and one
wide DVE multiply, split into two column halves that pipeline against each
other.  Rounds = L + OV + 1 = 81 instead of 512 serial steps.

The 65th stationary column of ones makes row 64 of each state the colsum of
the previous state; rows are captured (lagged, batched DMA) into a [81, 512]
history.  Stale renorm every W=16 rounds folds 1/colsum into an upcoming
emission tile (off the critical path) and books ln(colsum) into a per-window
history.  Host pre-computes exp(emissions) in bf16, arranged [round, k,
(chunk, b)], plus one-hot slot/window masks from the time mask; host also does
the final ln/stitch/sum assembly from small exported tensors.

Sharding: batch B=512 split across 8 cores (64 per core); transitions/alpha_0
replicated; final alpha_z = host sum of per-core assemblies.
"""

import os
import sys

for _p in ("/opt/trn_rl_repo", "/root/.axon_site/_ro/trn_rl_repo"):
    if os.path.isdir(_p) and _p not in sys.path:
        sys.path.insert(0, _p)

from contextlib import ExitStack

import numpy as np
import ml_dtypes

import concourse.bass as bass
import concourse.mybir as mybir
import concourse.tile as tile
from concourse.bass_utils import run_bass_kernel_spmd

# The walrus build in this container rejects instructions carrying more than
# one sync-wait command ("Too many sync wait commands" in setupSyncWait).
# Tile freely emits multi-wait instructions, so split the extras onto
# preceding same-engine no-ops at commit time (engine queues execute
# in-order, so the semantics are identical).
_ORIG_COMMIT = tile.TileContext._commit_instruction


def _single_wait_commit(self, inst, lazy_reg_writes=True):
    si = getattr(inst, "sync_info", None)
    if (
        si is not None
        and si.on_wait
        and len(si.on_wait) > 1
        and inst.engine != mybir.EngineType.Unassigned
    ):
        waits = list(si.on_wait)
        eng = self.nc.engines[inst.engine]
        for w in waits[:-1]:
            n = eng.nop(nofuse=True)
            n.ins.sync_info = mybir.SyncInfo(on_wait=[w], on_update=[])
        inst.sync_info = mybir.SyncInfo(
            on_wait=[waits[-1]], on_update=list(si.on_update or [])
        )
    _ORIG_COMMIT(self, inst, lazy_reg_writes)


tile.TileContext._commit_instruction = _single_wait_commit

T, B, K = 512, 512, 64
NCORES = 8
BSH = B // NCORES          # 64 batch columns per core
C = 8                      # time chunks run in parallel
L = T // C                 # 64 steps per chunk
OV = 16                    # burn-in overlap rounds
NR = L + OV + 1            # 81 rounds (round 80 exists only to capture cs_79)
W = 16                     # renorm window
NWC = 5                    # c_hist windows (0..4); folds at rounds 16/32/48/64
DELTA = 5.0
FW = C * BSH               # 512 fused columns, index = (chunk, b)
HWD = FW // 2              # half width for the two pipelined half-rounds
WRING = 16                 # state ring depth (capture DMA lags 8+ rounds)
ERING = 8                  # emission ring depth (prefetch 5 ahead)
F32 = mybir.dt.float32
BF16 = mybir.dt.bfloat16
MULT = mybir.AluOpType.mult
ADD = mybir.AluOpType.add
AX = mybir.AxisListType.X
AF = mybir.ActivationFunctionType
BF = ml_dtypes.bfloat16


def _build_crf_nc() -> bass.Bass:
    nc = bass.Bass(trn_type="TRN2", target_bir_lowering=False, debug=False)

    earr_d = nc.dram_tensor("earr", [NR, K, FW], BF16, kind="ExternalInput").ap()
    expal_d = nc.dram_tensor("expal", [K, 1], F32, kind="ExternalInput").ap()
    trans_d = nc.dram_tensor("transitions", [K, K], F32, kind="ExternalInput").ap()
    mslot_d = nc.dram_tensor("mslot", [NR, FW], BF16, kind="ExternalInput").ap()
    mwin_d = nc.dram_tensor("mwin", [NWC, FW], F32, kind="ExternalInput").ap()
    outa_d = nc.dram_tensor("outa", [2, BSH], F32, kind="ExternalOutput").ap()
    outb_d = nc.dram_tensor("outb", [2, FW], BF16, kind="ExternalOutput").ap()
    outc_d = nc.dram_tensor("outc", [1, FW], F32, kind="ExternalOutput").ap()

    with tile.TileContext(nc) as tc:
        with ExitStack() as ctx:
            _crf_body(ctx, tc, earr_d, expal_d, trans_d, mslot_d, mwin_d,
                      outa_d, outb_d, outc_d)
    _split_remaining_multiwaits(nc)
    return nc


def _split_remaining_multiwaits(nc):
    """Split multi-wait instructions added outside the commit path (e.g. the
    end-of-kernel drain/barrier) onto preceding same-engine no-ops."""
    for blk in nc.m.functions[0].blocks:
        il = blk.instructions
        idx = 0
        while idx < len(il):
            inst = il[idx]
            si = inst.sync_info
            if si is not None and si.on_wait and len(si.on_wait) > 1:
                waits = list(si.on_wait)
                for j, w in enumerate(waits[:-1]):
                    n = mybir.InstNoOp(
                        name=f"I-swx-{inst.name}-{j}", ins=[], outs=[]
                    )
                    n.engine = inst.engine
                    n.sync_info = mybir.SyncInfo(on_wait=[w], on_update=[])
                    nc.register_instruction(n, overwrite=True)
                    il.insert(idx, n)
                    idx += 1
                inst.sync_info = mybir.SyncInfo(
                    on_wait=[waits[-1]], on_update=list(si.on_update or [])
                )
            idx += 1


def _crf_body(ctx, tc, earr_d, expal_d, trans_d, mslot_d, mwin_d,
              outa_d, outb_d, outc_d):
    nc = tc.nc

    # ---- long-lived SBUF state ----
    ets = nc.alloc_sbuf_tensor("ets", [K, K + 1], BF16).ap()       # exp(tr-d)|1
    wring = nc.alloc_sbuf_tensor("wring", [K + 1, WRING * FW], BF16).ap()
    ering = nc.alloc_sbuf_tensor("ering", [K + 1, ERING * FW], BF16).ap()
    csum = nc.alloc_sbuf_tensor("csum", [NR, FW], BF16).ap()       # cs history
    c_rows = nc.alloc_sbuf_tensor("c_rows", [1, FW], F32).ap()     # Chat accum
    c_hist = nc.alloc_sbuf_tensor("c_hist", [NWC, FW], F32).ap()
    lns = nc.alloc_sbuf_tensor("lns", [1, FW], F32).ap()           # ln colsum
    rcr = nc.alloc_sbuf_tensor("rcr", [1, FW], BF16).ap()          # 1/colsum
    ones_r = nc.alloc_sbuf_tensor("ones_r", [1, K], BF16).ap()     # bcast stat
    expal_s = nc.alloc_sbuf_tensor("expal_s", [K, 1], F32).ap()
    mslot_s = nc.alloc_sbuf_tensor("mslot_s", [NR, FW], BF16).ap()
    mwin_s = nc.alloc_sbuf_tensor("mwin_s", [NWC, FW], F32).ap()
    ones_s = nc.alloc_sbuf_tensor("ones_s", [NR, 1], BF16).ap()    # part-reduce
    ones_w = nc.alloc_sbuf_tensor("ones_w", [NWC, 1], F32).ap()
    cst = nc.alloc_sbuf_tensor("cst", [K, 2], F32).ap()            # 0 | -DELTA

    ps_pool = ctx.enter_context(tc.tile_pool(name="ps", bufs=2, space="PSUM"))
    bc_pool = ctx.enter_context(tc.tile_pool(name="bc", bufs=1, space="PSUM"))
    fin_pool = ctx.enter_context(tc.tile_pool(name="fin", bufs=1))
    fps_pool = ctx.enter_context(tc.tile_pool(name="fps", bufs=2, space="PSUM"))

    def wsl(r, c0=0, c1=FW, p0=0, p1=K + 1):
        o = (r % WRING) * FW
        return wring[p0:p1, o + c0: o + c1]

    def esl(r, c0=0, c1=FW, p0=0, p1=K + 1):
        o = (r % ERING) * FW
        return ering[p0:p1, o + c0: o + c1]

    # ---- one-time setup ----
    nc.gpsimd.memset(cst[:, 0:1], 0.0)
    nc.gpsimd.memset(cst[:, 1:2], -DELTA)
    nc.gpsimd.memset(c_rows[:, :], 0.0)
    nc.gpsimd.memset(c_hist[:, :], 0.0)
    nc.vector.memset(wsl(-1, p1=K), 1.0)
    nc.vector.memset(
        ering.rearrange("p (s f) -> p s f", s=ERING)[K: K + 1, :, :], 1.0
    )
    nc.gpsimd.memset(ones_s[:, :], 1.0)
    nc.gpsimd.memset(ones_r[:, :], 1.0)
    nc.gpsimd.memset(ones_w[:, :], 1.0)

    tr_t = fin_pool.tile([K, K], F32)
    nc.sync.dma_start(tr_t[:], trans_d)
    nc.scalar.activation(ets[:, 0:K], tr_t[:], AF.Exp, bias=cst[0:K, 1:2])
    nc.vector.memset(ets[:, K: K + 1], 1.0)
    nc.sync.dma_start(expal_s, expal_d)
    nc.sync.dma_start(mslot_s, mslot_d)
    nc.sync.dma_start(mwin_s, mwin_d)
    for r in range(5):
        nc.sync.dma_start(esl(r, p1=K), earr_d[r])

    # ---- main loop: 81 fused rounds ----
    _rcb_live = [None]
    for r in range(NR):
        if r + 5 < NR:
            nc.sync.dma_start(esl(r + 5, p1=K), earr_d[r + 5])
        f = r + 2
        if f % W == 0 and W <= f <= 64:
            # renorm fold: scale the upcoming emission tile (DMA'd 3 rounds
            # ago, so the in-order DVE queue won't stall on it) by the stale
            # reciprocal-colsum.  Chunk 0 is excluded in window 1: its state
            # is exactly re-anchored at round OV and must not be scaled.
            c0 = BSH if f == W else 0
            nc.vector.tensor_tensor(
                esl(f, c0=c0, p1=K), esl(f, c0=c0, p1=K),
                _rcb_live[0][:, c0:], op=MULT,
            )
        for h in range(2):
            cl, cr_ = h * HWD, (h + 1) * HWD
            ps = ps_pool.tile([K + 1, HWD], F32, tag=f"ps{h}")
            nc.tensor.matmul(ps[:], ets[:, :], wsl(r - 1, cl, cr_, 0, K),
                             start=True, stop=True)
            nc.vector.tensor_tensor(
                wsl(r, cl, cr_), ps[:], esl(r, cl, cr_), op=MULT
            )
        if r == OV:
            # chunk 0 exact re-anchor: w = e_0 * exp(alpha_0), t = 0
            nc.vector.tensor_scalar(
                wsl(r, 0, BSH, 0, K), esl(r, 0, BSH, 0, K), expal_s, None,
                op0=MULT,
            )
            nc.gpsimd.memset(c_rows[:, 0:BSH], -DELTA * OV)
        if r % W == 4 and r < 64:
            # stale renorm prep from cs(state r-2) = row 64 of w_{r-1}
            nc.scalar.activation(lns, wsl(r - 1, p0=K), AF.Ln,
                                 bias=cst[0:1, 0:1])
            nc.scalar.activation(rcr, lns, AF.Exp, scale=-1.0,
                                 bias=cst[0:1, 0:1])
        if r % W == 8 and r < 64:
            # broadcast 1/colsum across partitions via PE outer product
            # (gpsimd partition_broadcast is rejected by this walrus build)
            rcb_ps = bc_pool.tile([K, FW], F32, tag="bc")
            nc.tensor.matmul(rcb_ps[:], ones_r, rcr, start=True, stop=True)
            _rcb_live[0] = rcb_ps
        if r % W == 5 and r < 64:
            # book ln(colsum) for this window (chunk-0 col reset at r=OV
            # lands between the window-0 booking and the c_hist[1] write)
            nc.gpsimd.tensor_tensor(c_rows, c_rows, lns, op=ADD)
        if r % W == 1 and W < r < 64 + W:
            nc.gpsimd.dma_start(c_hist[r // W: r // W + 1, :], c_rows)
        if r % 4 == 3 and r >= 11:
            s0 = r - 11
            nc.gpsimd.dma_start(
                csum[s0: s0 + 4, :],
                wring.rearrange("p (s f) -> p s f", s=WRING)[
                    K: K + 1, (s0 % WRING): (s0 % WRING) + 4, :
                ],
            )
    for s0, s1 in ((72, 76), (76, 80), (80, 81)):
        nc.gpsimd.dma_start(
            csum[s0:s1, :],
            wring.rearrange("p (s f) -> p s f", s=WRING)[
                K: K + 1, (s0 % WRING): (s0 % WRING) + (s1 - s0), :
            ],
        )

    # ---- final combine (small; host does ln/stitch) ----
    prod = fin_pool.tile([NR, FW], BF16, tag="prod")
    nc.vector.tensor_tensor(prod[:], csum[:, :], mslot_s[:, :], op=MULT)
    cs_ps = fps_pool.tile([1, FW], F32, tag="fps")
    nc.tensor.matmul(cs_ps[:], ones_s, prod[:], start=True, stop=True)
    prodw = fin_pool.tile([NWC, FW], F32, tag="prodw")
    nc.vector.tensor_tensor(prodw[:], c_hist[:, :], mwin_s[:, :], op=MULT)
    ch_ps = fps_pool.tile([1, FW], F32, tag="fps")
    nc.tensor.matmul(ch_ps[:], ones_w, prodw[:], start=True, stop=True)
    sel0 = fin_pool.tile([1, BSH], F32, tag="sel0")
    sel1 = fin_pool.tile([1, BSH], F32, tag="sel1")
    nc.vector.tensor_reduce(
        sel0[:], cs_ps[:].rearrange("o (c b) -> o b c", c=C), axis=AX, op=ADD
    )
    nc.vector.tensor_reduce(
        sel1[:], ch_ps[:].rearrange("o (c b) -> o b c", c=C), axis=AX, op=ADD
    )
    nc.sync.dma_start(outa_d[0:1, :], sel0[:])
    nc.sync.dma_start(outa_d[1:2, :], sel1[:])
    nc.sync.dma_start(outb_d[0:1, :], csum[NR - 1: NR, :])
    nc.sync.dma_start(outb_d[1:2, :], csum[OV: OV + 1, :])
    nc.sync.dma_start(outc_d, c_hist[NWC - 1: NWC, :])


_NC_CACHE = None


def _get_nc():
    global _NC_CACHE
    if _NC_CACHE is None:
        _NC_CACHE = _build_crf_nc()
    return _NC_CACHE


def _prep(np_inputs):
    """Host-side input prep: per-core arranged/pre-exp'd emissions + masks.

    Returns (in_maps, aux) where aux carries per-core (i_b, c_b) for the
    host-side assembly."""
    emits = np.asarray(np_inputs["emits"], dtype=np.float32)
    mask = np.asarray(np_inputs["mask"])
    transitions = np.asarray(np_inputs["transitions"], dtype=np.float32)
    alpha_0 = np.asarray(np_inputs["alpha_0"], dtype=np.float32)
    tau_all = np.argmax(mask, axis=0).astype(np.int64)  # [B]
    expal = np.exp(alpha_0).astype(np.float32)

    in_maps, aux = [], []
    for n in range(NCORES):
        sl = slice(n * BSH, (n + 1) * BSH)
        # padded exp(emits) [529, K, BSH]: P[t+OV] = exp(emits[t])^T
        pad = np.ones((T + OV + NR - L - OV, K, BSH), dtype=np.float32)
        pad[OV: OV + T] = np.exp(emits[:, sl, :]).transpose(0, 2, 1)
        idx = np.arange(NR)[:, None] + np.arange(C)[None, :] * L  # [NR, C]
        earr = pad[idx]                        # [NR, C, K, BSH]
        earr = earr.transpose(0, 2, 1, 3).reshape(NR, K, FW).astype(BF)

        tau = tau_all[sl]
        cb = tau // L
        ib = tau - cb * L + OV
        col = cb * BSH + np.arange(BSH)
        mslot = np.zeros((NR, FW), dtype=BF)
        mslot[ib + 1, col] = 1.0
        mwin = np.zeros((NWC, FW), dtype=np.float32)
        mwin[ib // W, col] = 1.0
        in_maps.append(
            {
                "earr": earr,
                "expal": expal,
                "transitions": transitions,
                "mslot": mslot,
                "mwin": mwin,
            }
        )
        aux.append((ib, cb))
    return in_maps, aux


def _assemble(results, aux):
    """Host-side final assembly: ln, chunk-scale stitch, and global sum."""
    total = np.float64(0.0)
    for res, (ib, cb) in zip(results, aux):
        cs_sel = np.asarray(res["outa"][0], dtype=np.float64)
        ch_sel = np.asarray(res["outa"][1], dtype=np.float64)
        csR = np.asarray(res["outb"][0], dtype=np.float64).reshape(C, BSH)
        csOV = np.asarray(res["outb"][1], dtype=np.float64).reshape(C, BSH)
        chR = np.asarray(res["outc"][0], dtype=np.float64).reshape(C, BSH)
        d = (np.log(csR[:-1]) + chR[:-1] + DELTA * (NR - 2)) - (
            np.log(csOV[1:]) + DELTA * (OV - 1)
        )
        lam = np.concatenate(
            [np.zeros((1, BSH)), np.cumsum(d, axis=0)], axis=0
        )  # [C, BSH]
        b = np.arange(BSH)
        r = np.log(cs_sel) + ch_sel + DELTA * ib + lam[cb, b]
        total += r.sum()
    return np.float32(total)


def kernel(emits, mask, transitions, alpha_0):
    nc = _get_nc()
    in_maps, aux = _prep(
        {"emits": emits, "mask": mask, "transitions": transitions,
         "alpha_0": alpha_0}
    )
    res = run_bass_kernel_spmd(nc, in_maps, core_ids=list(range(NCORES)))
    return _assemble(res.results, aux)


# revision 15
# speedup vs baseline: 3.8904x; 1.4997x over previous
"""CRF forward (logsumexp over paths) loss kernel for Trainium2, 8 NeuronCores.

Chunk-parallel-in-time formulation
----------------------------------
reference:  fv0 = alpha_0^T + emits[0]                       [B, K]
            fv_t[b,j] = logsumexp_i(fv_{t-1}[b,i] + trans[i,j]) + emit_t[b,j]
            alpha_z = sum_b logsumexp_k( fv_{tau_b}[b,:] )   (tau = one-hot mask step)

In exp space the recurrence w_t = (ETs^T w_{t-1}) * e_t (ETs = exp(trans-DELTA))
is a product of strictly positive matrices, which contracts any two initial
states to the same *direction* at ~1e-1/step (Birkhoff).  So the time axis is
split into C=8 chunks of L=64 steps run CONCURRENTLY, each started from an
all-ones guess OV=16 steps early; after the burn-in the chunk states equal the
true states up to a per-column scalar.  That scalar is recovered exactly by
comparing log-colsums of adjacent chunks at the overlap boundary (a length-8
prefix sum done on the host from exported colsums).

The C concurrent chunks fuse into WIDE instructions: per round one
[64->65, 512] matmul (stationary never changes -> no LDWEIGHTS churn) and one
wide DVE multiply, split into two column halves that pipeline against each
other.  Rounds = L + OV + 1 = 81 instead of 512 serial steps.

The 65th stationary column of ones makes row 64 of each state the colsum of
the previous state; rows are captured (lagged, batched DMA) into a [81, 512]
history.  Stale renorm every W=16 rounds folds 1/colsum into an upcoming
emission tile (off the critical path) and books ln(colsum) into a per-window
history.  Host pre-computes exp(emissions) in bf16, arranged [round, k,
(chunk, b)], plus one-hot slot/window masks from the time mask; host also does
the final ln/stitch/sum assembly from small exported tensors.

Sharding: batch B=512 split across 8 cores (64 per core); transitions/alpha_0
replicated; final alpha_z = host sum of per-core assemblies.
"""

import os
import sys

for _p in ("/opt/trn_rl_repo", "/root/.axon_site/_ro/trn_rl_repo"):
    if os.path.isdir(_p) and _p not in sys.path:
        sys.path.insert(0, _p)

from contextlib import ExitStack

import numpy as np
import ml_dtypes

import concourse.bass as bass
import concourse.mybir as mybir
import concourse.tile as tile
from concourse.bass_utils import run_bass_kernel_spmd

# The walrus build in this container rejects instructions carrying more than
# one sync-wait command ("Too many sync wait commands" in setupSyncWait).
# Tile freely emits multi-wait instructions, so split the extras onto
# preceding same-engine no-ops at commit time (engine queues execute
# in-order, so the semantics are identical).
_ORIG_COMMIT = tile.TileContext._commit_instruction


def _single_wait_commit(self, inst, lazy_reg_writes=True):
    si = getattr(inst, "sync_info", None)
    if (
        si is not None
        and si.on_wait
        and len(si.on_wait) > 1
        and inst.engine != mybir.EngineType.Unassigned
    ):
        waits = list(si.on_wait)
        eng = self.nc.engines[inst.engine]
        for w in waits[:-1]:
            n = eng.nop(nofuse=True)
            n.ins.sync_info = mybir.SyncInfo(on_wait=[w], on_update=[])
        inst.sync_info = mybir.SyncInfo(
            on_wait=[waits[-1]], on_update=list(si.on_update or [])
        )
    _ORIG_COMMIT(self, inst, lazy_reg_writes)


tile.TileContext._commit_instruction = _single_wait_commit

T, B, K = 512, 512, 64
NCORES = 8
BSH = B // NCORES          # 64 batch columns per core
C = 16                     # time chunks run in parallel
L = T // C                 # 32 steps per chunk
OV = 8                     # burn-in overlap rounds
NR = L + OV + 1            # 41 rounds (last round exists only to capture cs)
W = 16                     # renorm window
NFOLD = (NR - 2) // W      # renorm folds, at rounds W, 2W, ...
NWC = NFOLD + 1            # c_hist windows
DELTA = 5.0
FW = C * BSH               # 512 fused columns, index = (chunk, b)
HWD = FW // 2              # half width for the two pipelined half-rounds
WRING = 16                 # state ring depth (capture DMA lags 8+ rounds)
ERING = 8                  # emission ring depth (prefetch 5 ahead)
F32 = mybir.dt.float32
BF16 = mybir.dt.bfloat16
MULT = mybir.AluOpType.mult
ADD = mybir.AluOpType.add
AX = mybir.AxisListType.X
AF = mybir.ActivationFunctionType
BF = ml_dtypes.bfloat16


def _build_crf_nc() -> bass.Bass:
    nc = bass.Bass(trn_type="TRN2", target_bir_lowering=False, debug=False)

    earr_d = nc.dram_tensor("earr", [NR, K + 1, FW], BF16, kind="ExternalInput").ap()
    expal_d = nc.dram_tensor("expal", [K, 1], F32, kind="ExternalInput").ap()
    trans_d = nc.dram_tensor("transitions", [K, K], F32, kind="ExternalInput").ap()
    csum_d = nc.dram_tensor("csum_out", [NR, FW], BF16, kind="ExternalOutput").ap()
    chist_d = nc.dram_tensor("chist_out", [NWC, FW], F32, kind="ExternalOutput").ap()

    with tile.TileContext(nc) as tc:
        with ExitStack() as ctx:
            _crf_body(ctx, tc, earr_d, expal_d, trans_d, csum_d, chist_d)
    _split_remaining_multiwaits(nc)
    return nc


def _split_remaining_multiwaits(nc):
    """Split multi-wait instructions added outside the commit path (e.g. the
    end-of-kernel drain/barrier) onto preceding same-engine no-ops."""
    for blk in nc.m.functions[0].blocks:
        il = blk.instructions
        idx = 0
        while idx < len(il):
            inst = il[idx]
            si = inst.sync_info
            if si is not None and si.on_wait and len(si.on_wait) > 1:
                waits = list(si.on_wait)
                for j, w in enumerate(waits[:-1]):
                    n = mybir.InstNoOp(
                        name=f"I-swx-{inst.name}-{j}", ins=[], outs=[]
                    )
                    n.engine = inst.engine
                    n.sync_info = mybir.SyncInfo(on_wait=[w], on_update=[])
                    nc.register_instruction(n, overwrite=True)
                    il.insert(idx, n)
                    idx += 1
                inst.sync_info = mybir.SyncInfo(
                    on_wait=[waits[-1]], on_update=list(si.on_update or [])
                )
            idx += 1


def _crf_body(ctx, tc, earr_d, expal_d, trans_d, csum_d, chist_d):
    nc = tc.nc

    # ---- long-lived SBUF state ----
    ets = nc.alloc_sbuf_tensor("ets", [K, K + 1], BF16).ap()       # exp(tr-d)|1
    wring = nc.alloc_sbuf_tensor("wring", [K + 1, WRING * FW], BF16).ap()
    ering = nc.alloc_sbuf_tensor("ering", [K + 1, ERING * FW], BF16).ap()
    csum = nc.alloc_sbuf_tensor("csum", [NR, FW], BF16).ap()       # cs history
    c_rows = nc.alloc_sbuf_tensor("c_rows", [1, FW], F32).ap()     # Chat accum
    c_hist = nc.alloc_sbuf_tensor("c_hist", [NWC, FW], F32).ap()
    lns = nc.alloc_sbuf_tensor("lns", [1, FW], F32).ap()           # ln colsum
    rcr = nc.alloc_sbuf_tensor("rcr", [1, FW], BF16).ap()          # 1/colsum
    ones_r = nc.alloc_sbuf_tensor("ones_r", [1, K], BF16).ap()     # bcast stat
    expal_s = nc.alloc_sbuf_tensor("expal_s", [K, 1], F32).ap()
    cst = nc.alloc_sbuf_tensor("cst", [K, 2], F32).ap()            # 0 | -DELTA

    ps_pool = ctx.enter_context(tc.tile_pool(name="ps", bufs=2, space="PSUM"))
    bc_pool = ctx.enter_context(tc.tile_pool(name="bc", bufs=1, space="PSUM"))

    def wsl(r, c0=0, c1=FW, p0=0, p1=K + 1):
        o = (r % WRING) * FW
        return wring[p0:p1, o + c0: o + c1]

    def esl(r, c0=0, c1=FW, p0=0, p1=K + 1):
        o = (r % ERING) * FW
        return ering[p0:p1, o + c0: o + c1]

    # ---- one-time setup ----
    nc.gpsimd.memset(cst[:, 0:1], 0.0)
    nc.gpsimd.memset(cst[:, 1:2], -DELTA)
    nc.gpsimd.memset(c_rows[:, :], 0.0)
    nc.gpsimd.memset(c_hist[:, :], 0.0)
    nc.vector.memset(wsl(-1, p1=K), 1.0)
    nc.gpsimd.memset(ones_r[:, :], 1.0)

    tr_t = nc.alloc_sbuf_tensor("tr_t", [K, K], F32).ap()
    nc.sync.dma_start(tr_t, trans_d)
    nc.scalar.activation(ets[:, 0:K], tr_t, AF.Exp, bias=cst[0:K, 1:2])
    nc.vector.memset(ets[:, K: K + 1], 1.0)
    nc.gpsimd.dma_start(expal_s, expal_d)
    for r in range(5):
        nc.sync.dma_start(esl(r), earr_d[r])

    # ---- main loop: 81 fused rounds ----
    _rcb_live = [None, None]
    for r in range(NR):
        if r + 5 < NR:
            nc.sync.dma_start(esl(r + 5), earr_d[r + 5])
        f = r + 2
        if f % W == 0 and W <= f <= NFOLD * W:
            # renorm fold: scale the upcoming emission tile (DMA'd 3 rounds
            # ago, so the in-order DVE queue won't stall on it) by the stale
            # reciprocal-colsum.  Chunk 0 is excluded in window 1: its state
            # is exactly re-anchored at round OV and must not be scaled.
            for h in range(2):
                c0 = max(BSH if f == W else 0, h * HWD)
                if c0 >= (h + 1) * HWD:
                    continue
                nc.vector.tensor_tensor(
                    esl(f, c0=c0, c1=(h + 1) * HWD, p1=K),
                    esl(f, c0=c0, c1=(h + 1) * HWD, p1=K),
                    _rcb_live[h][:, c0 - h * HWD:], op=MULT,
                )
        for h in range(2):
            cl, cr_ = h * HWD, (h + 1) * HWD
            ps = ps_pool.tile([K + 1, HWD], F32, tag=f"ps{h}")
            nc.tensor.matmul(ps[:], ets[:, :], wsl(r - 1, cl, cr_, 0, K),
                             start=True, stop=True)
            nc.vector.tensor_tensor(
                wsl(r, cl, cr_), ps[:], esl(r, cl, cr_), op=MULT
            )
        if r == OV:
            # chunk 0 exact re-anchor: w = e_0 * exp(alpha_0), t = 0
            nc.vector.tensor_scalar(
                wsl(r, 0, BSH, 0, K), esl(r, 0, BSH, 0, K), expal_s, None,
                op0=MULT,
            )
            nc.gpsimd.memset(c_rows[:, 0:BSH], -DELTA * OV)
        if r % W == 4 and r // W < NFOLD:
            # stale renorm prep from cs(state r-2) = row 64 of w_{r-1}
            nc.scalar.activation(lns, wsl(r - 1, p0=K), AF.Ln,
                                 bias=cst[0:1, 0:1])
            nc.scalar.activation(rcr, lns, AF.Exp, scale=-1.0,
                                 bias=cst[0:1, 0:1])
        if r % W == 8 and r // W < NFOLD:
            # broadcast 1/colsum across partitions via PE outer products
            # (gpsimd partition_broadcast is rejected by this walrus build;
            # split in half: a [K, FW] fp32 output would span two PSUM banks)
            for h in range(2):
                rcb_ps = bc_pool.tile([K, HWD], F32, tag=f"bc{h}")
                nc.tensor.matmul(rcb_ps[:], ones_r, rcr[:, h * HWD:(h + 1) * HWD],
                                 start=True, stop=True)
                _rcb_live[h] = rcb_ps
        if r % W == 5 and r // W < NFOLD:
            # book ln(colsum) for this window (chunk-0 col reset at r=OV
            # lands between the window-0 booking and the c_hist[1] write)
            nc.gpsimd.tensor_tensor(c_rows, c_rows, lns, op=ADD)
        if r % W == 1 and 1 <= r // W <= NFOLD:
            nc.gpsimd.dma_start(c_hist[r // W: r // W + 1, :], c_rows)
        if r % 4 == 3 and r >= 11:
            s0 = r - 11
            nc.gpsimd.dma_start(
                csum[s0: s0 + 4, :],
                wring.rearrange("p (s f) -> p s f", s=WRING)[
                    K: K + 1, (s0 % WRING): (s0 % WRING) + 4, :
                ],
            )
    _pstart = ((NR - 1 - 11) // 4) * 4 + 4   # first slot not captured in-loop
    _groups = [(a, min(a + 4, NR)) for a in range(_pstart, NR, 4)]
    for s0, s1 in _groups:
        nc.gpsimd.dma_start(
            csum[s0:s1, :],
            wring.rearrange("p (s f) -> p s f", s=WRING)[
                K: K + 1, (s0 % WRING): (s0 % WRING) + (s1 - s0), :
            ],
        )

    # ---- export raw histories; host does select/ln/stitch ----
    nc.sync.dma_start(csum_d, csum[:, :])
    nc.sync.dma_start(chist_d, c_hist[:, :])


_NC_CACHE = None


def _get_nc():
    global _NC_CACHE
    if _NC_CACHE is None:
        _NC_CACHE = _build_crf_nc()
    return _NC_CACHE


def _prep(np_inputs):
    """Host-side input prep: per-core arranged/pre-exp'd emissions + masks.

    Returns (in_maps, aux) where aux carries per-core (i_b, c_b) for the
    host-side assembly."""
    emits = np.asarray(np_inputs["emits"], dtype=np.float32)
    mask = np.asarray(np_inputs["mask"])
    transitions = np.asarray(np_inputs["transitions"], dtype=np.float32)
    alpha_0 = np.asarray(np_inputs["alpha_0"], dtype=np.float32)
    tau_all = np.argmax(mask, axis=0).astype(np.int64)  # [B]
    expal = np.exp(alpha_0).astype(np.float32)

    in_maps, aux = [], []
    for n in range(NCORES):
        sl = slice(n * BSH, (n + 1) * BSH)
        # padded exp(emits) [529, K, BSH]: P[t+OV] = exp(emits[t])^T
        pad = np.ones((T + OV + NR - L - OV, K, BSH), dtype=np.float32)
        pad[OV: OV + T] = np.exp(emits[:, sl, :]).transpose(0, 2, 1)
        idx = np.arange(NR)[:, None] + np.arange(C)[None, :] * L  # [NR, C]
        earr = pad[idx]                        # [NR, C, K, BSH]
        earr = earr.transpose(0, 2, 1, 3).reshape(NR, K, FW)
        earr = np.concatenate(
            [earr, np.ones((NR, 1, FW), np.float32)], axis=1
        ).astype(BF)                           # row 64 = tt passthrough ones

        tau = tau_all[sl]
        cb = tau // L
        ib = tau - cb * L + OV
        in_maps.append(
            {"earr": earr, "expal": expal, "transitions": transitions}
        )
        aux.append((ib, cb))
    return in_maps, aux


def _assemble(results, aux):
    """Host-side final assembly: ln, chunk-scale stitch, and global sum."""
    total = np.float64(0.0)
    for res, (ib, cb) in zip(results, aux):
        csum = np.asarray(res["csum_out"], dtype=np.float64).reshape(NR, C, BSH)
        chist = np.asarray(res["chist_out"], dtype=np.float64).reshape(
            NWC, C, BSH
        )
        b = np.arange(BSH)
        cs_sel = csum[ib + 1, cb, b]
        ch_sel = chist[ib // W, cb, b]
        # chunk 0's exact re-anchor books -DELTA*OV into c_rows at round OV;
        # window-0 states (no fold yet) read c_hist[0]=0, so patch it here
        ch_sel = np.where((cb == 0) & (ib // W == 0), -DELTA * OV, ch_sel)
        csR, csOV, chR = csum[NR - 1], csum[OV], chist[NWC - 1]
        d = (np.log(csR[:-1]) + chR[:-1] + DELTA * (NR - 2)) - (
            np.log(csOV[1:]) + DELTA * (OV - 1)
        )
        lam = np.concatenate(
            [np.zeros((1, BSH)), np.cumsum(d, axis=0)], axis=0
        )  # [C, BSH]
        r = np.log(cs_sel) + ch_sel + DELTA * ib + lam[cb, b]
        total += r.sum()
    return np.float32(total)


def kernel(emits, mask, transitions, alpha_0):
    nc = _get_nc()
    in_maps, aux = _prep(
        {"emits": emits, "mask": mask, "transitions": transitions,
         "alpha_0": alpha_0}
    )
    res = run_bass_kernel_spmd(nc, in_maps, core_ids=list(range(NCORES)))
    return _assemble(res.results, aux)


# revision 16
# speedup vs baseline: 4.2837x; 1.1011x over previous
"""CRF forward (logsumexp over paths) loss kernel for Trainium2, 8 NeuronCores.

Chunk-parallel-in-time formulation
----------------------------------
reference:  fv0 = alpha_0^T + emits[0]                       [B, K]
            fv_t[b,j] = logsumexp_i(fv_{t-1}[b,i] + trans[i,j]) + emit_t[b,j]
            alpha_z = sum_b logsumexp_k( fv_{tau_b}[b,:] )   (tau = one-hot mask step)

In exp space the recurrence w_t = (ETs^T w_{t-1}) * e_t (ETs = exp(trans-DELTA))
is a product of strictly positive matrices, which contracts any two initial
states to the same *direction* at ~1e-1/step (Birkhoff).  So the time axis is
split into C=8 chunks of L=64 steps run CONCURRENTLY, each started from an
all-ones guess OV=16 steps early; after the burn-in the chunk states equal the
true states up to a per-column scalar.  That scalar is recovered exactly by
comparing log-colsums of adjacent chunks at the overlap boundary (a length-8
prefix sum done on the host from exported colsums).

The C concurrent chunks fuse into WIDE instructions: per round one
[64->65, 512] matmul (stationary never changes -> no LDWEIGHTS churn) and one
wide DVE multiply, split into two column halves that pipeline against each
other.  Rounds = L + OV + 1 = 81 instead of 512 serial steps.

The 65th stationary column of ones makes row 64 of each state the colsum of
the previous state; rows are captured (lagged, batched DMA) into a [81, 512]
history.  Stale renorm every W=16 rounds folds 1/colsum into an upcoming
emission tile (off the critical path) and books ln(colsum) into a per-window
history.  Host pre-computes exp(emissions) in bf16, arranged [round, k,
(chunk, b)], plus one-hot slot/window masks from the time mask; host also does
the final ln/stitch/sum assembly from small exported tensors.

Sharding: batch B=512 split across 8 cores (64 per core); transitions/alpha_0
replicated; final alpha_z = host sum of per-core assemblies.
"""

import os
import sys

for _p in ("/opt/trn_rl_repo", "/root/.axon_site/_ro/trn_rl_repo"):
    if os.path.isdir(_p) and _p not in sys.path:
        sys.path.insert(0, _p)

from contextlib import ExitStack

import numpy as np
import ml_dtypes

import concourse.bass as bass
import concourse.mybir as mybir
import concourse.tile as tile
from concourse.bass_utils import run_bass_kernel_spmd

# The walrus build in this container rejects instructions carrying more than
# one sync-wait command ("Too many sync wait commands" in setupSyncWait).
# Tile freely emits multi-wait instructions, so split the extras onto
# preceding same-engine no-ops at commit time (engine queues execute
# in-order, so the semantics are identical).
_ORIG_COMMIT = tile.TileContext._commit_instruction


def _single_wait_commit(self, inst, lazy_reg_writes=True):
    si = getattr(inst, "sync_info", None)
    if (
        si is not None
        and si.on_wait
        and len(si.on_wait) > 1
        and inst.engine != mybir.EngineType.Unassigned
    ):
        waits = list(si.on_wait)
        eng = self.nc.engines[inst.engine]
        for w in waits[:-1]:
            n = eng.nop(nofuse=True)
            n.ins.sync_info = mybir.SyncInfo(on_wait=[w], on_update=[])
        inst.sync_info = mybir.SyncInfo(
            on_wait=[waits[-1]], on_update=list(si.on_update or [])
        )
    _ORIG_COMMIT(self, inst, lazy_reg_writes)


tile.TileContext._commit_instruction = _single_wait_commit

T, B, K = 512, 512, 64
NCORES = 8
BSH = B // NCORES          # 64 batch columns per core
C = 16                     # time chunks run in parallel
L = T // C                 # 32 steps per chunk
OV = 6                     # burn-in overlap rounds
NR = L + OV + 1            # 39 rounds (last round exists only to capture cs)
W = 32                     # renorm window
NFOLD = (NR - 2) // W      # renorm folds, at rounds W, 2W, ...
NWC = NFOLD + 1            # c_hist windows
DELTA = 5.0
FW = C * BSH               # 512 fused columns, index = (chunk, b)
HWD = FW // 2              # half width for the two pipelined half-rounds
WRING = 16                 # state ring depth (capture DMA lags 8+ rounds)
ERING = 8                  # emission ring depth (prefetch 5 ahead)
F32 = mybir.dt.float32
BF16 = mybir.dt.bfloat16
MULT = mybir.AluOpType.mult
ADD = mybir.AluOpType.add
AX = mybir.AxisListType.X
AF = mybir.ActivationFunctionType
BF = ml_dtypes.bfloat16


def _build_crf_nc() -> bass.Bass:
    nc = bass.Bass(trn_type="TRN2", target_bir_lowering=False, debug=False)

    earr_d = nc.dram_tensor("earr", [NR, K + 1, FW], BF16, kind="ExternalInput").ap()
    expal_d = nc.dram_tensor("expal", [K, 1], F32, kind="ExternalInput").ap()
    trans_d = nc.dram_tensor("transitions", [K, K], F32, kind="ExternalInput").ap()
    csum_d = nc.dram_tensor("csum_out", [NR, FW], BF16, kind="ExternalOutput").ap()
    chist_d = nc.dram_tensor("chist_out", [NWC, FW], F32, kind="ExternalOutput").ap()

    with tile.TileContext(nc) as tc:
        with ExitStack() as ctx:
            _crf_body(ctx, tc, earr_d, expal_d, trans_d, csum_d, chist_d)
    _split_remaining_multiwaits(nc)
    return nc


def _split_remaining_multiwaits(nc):
    """Split multi-wait instructions added outside the commit path (e.g. the
    end-of-kernel drain/barrier) onto preceding same-engine no-ops."""
    for blk in nc.m.functions[0].blocks:
        il = blk.instructions
        idx = 0
        while idx < len(il):
            inst = il[idx]
            si = inst.sync_info
            if si is not None and si.on_wait and len(si.on_wait) > 1:
                waits = list(si.on_wait)
                for j, w in enumerate(waits[:-1]):
                    n = mybir.InstNoOp(
                        name=f"I-swx-{inst.name}-{j}", ins=[], outs=[]
                    )
                    n.engine = inst.engine
                    n.sync_info = mybir.SyncInfo(on_wait=[w], on_update=[])
                    nc.register_instruction(n, overwrite=True)
                    il.insert(idx, n)
                    idx += 1
                inst.sync_info = mybir.SyncInfo(
                    on_wait=[waits[-1]], on_update=list(si.on_update or [])
                )
            idx += 1


def _crf_body(ctx, tc, earr_d, expal_d, trans_d, csum_d, chist_d):
    nc = tc.nc

    # ---- long-lived SBUF state ----
    ets = nc.alloc_sbuf_tensor("ets", [K, K + 1], BF16).ap()       # exp(tr-d)|1
    wring = nc.alloc_sbuf_tensor("wring", [K + 1, WRING * FW], BF16).ap()
    ering = nc.alloc_sbuf_tensor("ering", [K + 1, ERING * FW], BF16).ap()
    csum = nc.alloc_sbuf_tensor("csum", [NR, FW], BF16).ap()       # cs history
    c_rows = nc.alloc_sbuf_tensor("c_rows", [1, FW], F32).ap()     # Chat accum
    c_hist = nc.alloc_sbuf_tensor("c_hist", [NWC, FW], F32).ap()
    lns = nc.alloc_sbuf_tensor("lns", [1, FW], F32).ap()           # ln colsum
    rcr = nc.alloc_sbuf_tensor("rcr", [1, FW], BF16).ap()          # 1/colsum
    ones_r = nc.alloc_sbuf_tensor("ones_r", [1, K], BF16).ap()     # bcast stat
    expal_s = nc.alloc_sbuf_tensor("expal_s", [K, 1], F32).ap()
    cst = nc.alloc_sbuf_tensor("cst", [K, 2], F32).ap()            # 0 | -DELTA

    ps_pool = ctx.enter_context(tc.tile_pool(name="ps", bufs=2, space="PSUM"))
    bc_pool = ctx.enter_context(tc.tile_pool(name="bc", bufs=1, space="PSUM"))

    def wsl(r, c0=0, c1=FW, p0=0, p1=K + 1):
        o = (r % WRING) * FW
        return wring[p0:p1, o + c0: o + c1]

    def esl(r, c0=0, c1=FW, p0=0, p1=K + 1):
        o = (r % ERING) * FW
        return ering[p0:p1, o + c0: o + c1]

    # ---- one-time setup ----
    nc.gpsimd.memset(cst[:, 0:1], 0.0)
    nc.gpsimd.memset(cst[:, 1:2], -DELTA)
    nc.gpsimd.memset(c_rows[:, :], 0.0)
    nc.gpsimd.memset(c_hist[:, :], 0.0)
    nc.vector.memset(wsl(-1, p1=K), 1.0)
    nc.gpsimd.memset(ones_r[:, :], 1.0)

    tr_t = nc.alloc_sbuf_tensor("tr_t", [K, K], F32).ap()
    nc.sync.dma_start(tr_t, trans_d)
    nc.scalar.activation(ets[:, 0:K], tr_t, AF.Exp, bias=cst[0:K, 1:2])
    nc.vector.memset(ets[:, K: K + 1], 1.0)
    nc.gpsimd.dma_start(expal_s, expal_d)
    for r in range(5):
        nc.sync.dma_start(esl(r), earr_d[r])

    # ---- main loop: 81 fused rounds ----
    _rcb_live = [None, None]
    for r in range(NR):
        if r + 5 < NR:
            nc.sync.dma_start(esl(r + 5), earr_d[r + 5])
        f = r + 2
        if f % W == 0 and W <= f <= NFOLD * W:
            # renorm fold: scale the upcoming emission tile (DMA'd 3 rounds
            # ago, so the in-order DVE queue won't stall on it) by the stale
            # reciprocal-colsum.  Chunk 0 is excluded in window 1: its state
            # is exactly re-anchored at round OV and must not be scaled.
            for h in range(2):
                c0 = max(BSH if f == W else 0, h * HWD)
                if c0 >= (h + 1) * HWD:
                    continue
                nc.vector.tensor_tensor(
                    esl(f, c0=c0, c1=(h + 1) * HWD, p1=K),
                    esl(f, c0=c0, c1=(h + 1) * HWD, p1=K),
                    _rcb_live[h][:, c0 - h * HWD:], op=MULT,
                )
        for h in range(2):
            cl, cr_ = h * HWD, (h + 1) * HWD
            ps = ps_pool.tile([K + 1, HWD], F32, tag=f"ps{h}")
            nc.tensor.matmul(ps[:], ets[:, :], wsl(r - 1, cl, cr_, 0, K),
                             start=True, stop=True)
            nc.vector.tensor_tensor(
                wsl(r, cl, cr_), ps[:], esl(r, cl, cr_), op=MULT
            )
        if r == OV:
            # chunk 0 exact re-anchor: w = e_0 * exp(alpha_0), t = 0
            nc.vector.tensor_scalar(
                wsl(r, 0, BSH, 0, K), esl(r, 0, BSH, 0, K), expal_s, None,
                op0=MULT,
            )
            nc.gpsimd.memset(c_rows[:, 0:BSH], -DELTA * OV)
        if r % W == 4 and r // W < NFOLD:
            # stale renorm prep from cs(state r-2) = row 64 of w_{r-1}
            nc.scalar.activation(lns, wsl(r - 1, p0=K), AF.Ln,
                                 bias=cst[0:1, 0:1])
            nc.scalar.activation(rcr, lns, AF.Exp, scale=-1.0,
                                 bias=cst[0:1, 0:1])
        if r % W == 8 and r // W < NFOLD:
            # broadcast 1/colsum across partitions via PE outer products
            # (gpsimd partition_broadcast is rejected by this walrus build;
            # split in half: a [K, FW] fp32 output would span two PSUM banks)
            for h in range(2):
                rcb_ps = bc_pool.tile([K, HWD], F32, tag=f"bc{h}")
                nc.tensor.matmul(rcb_ps[:], ones_r, rcr[:, h * HWD:(h + 1) * HWD],
                                 start=True, stop=True)
                _rcb_live[h] = rcb_ps
        if r % W == 5 and r // W < NFOLD:
            # book ln(colsum) for this window (chunk-0 col reset at r=OV
            # lands between the window-0 booking and the c_hist[1] write)
            nc.gpsimd.tensor_tensor(c_rows, c_rows, lns, op=ADD)
        if r % W == 1 and 1 <= r // W <= NFOLD:
            nc.gpsimd.dma_start(c_hist[r // W: r // W + 1, :], c_rows)
        caps = [r - 11] if r % 4 == 3 and r >= 11 else []
        if r == NR - 4:
            # catch-up: slots up to r are final; lag-4 is still WAR-safe
            # (ring position r%WRING is not rewritten before the loop ends)
            caps += [r - 7, r - 3]
        for s0 in caps:
            nc.gpsimd.dma_start(
                csum[s0: s0 + 4, :],
                wring.rearrange("p (s f) -> p s f", s=WRING)[
                    K: K + 1, (s0 % WRING): (s0 % WRING) + 4, :
                ],
            )
        if r == NR - 3:
            # bulk colsum export overlaps the last rounds; the tail only
            # ships the final 3 slots + c_hist
            nc.sync.dma_start(csum_d[0: NR - 3, :], csum[0: NR - 3, :])
    s0 = NR - 3
    nc.gpsimd.dma_start(
        csum[s0:NR, :],
        wring.rearrange("p (s f) -> p s f", s=WRING)[
            K: K + 1, (s0 % WRING): (s0 % WRING) + (NR - s0), :
        ],
    )

    # ---- export the tail of the histories; host does select/ln/stitch ----
    nc.sync.dma_start(csum_d[NR - 3: NR, :], csum[NR - 3: NR, :])
    nc.sync.dma_start(chist_d, c_hist[:, :])


_NC_CACHE = None


def _get_nc():
    global _NC_CACHE
    if _NC_CACHE is None:
        _NC_CACHE = _build_crf_nc()
    return _NC_CACHE


def _prep(np_inputs):
    """Host-side input prep: per-core arranged/pre-exp'd emissions + masks.

    Returns (in_maps, aux) where aux carries per-core (i_b, c_b) for the
    host-side assembly."""
    emits = np.asarray(np_inputs["emits"], dtype=np.float32)
    mask = np.asarray(np_inputs["mask"])
    transitions = np.asarray(np_inputs["transitions"], dtype=np.float32)
    alpha_0 = np.asarray(np_inputs["alpha_0"], dtype=np.float32)
    tau_all = np.argmax(mask, axis=0).astype(np.int64)  # [B]
    expal = np.exp(alpha_0).astype(np.float32)

    in_maps, aux = [], []
    for n in range(NCORES):
        sl = slice(n * BSH, (n + 1) * BSH)
        # padded exp(emits) [529, K, BSH]: P[t+OV] = exp(emits[t])^T
        pad = np.ones((T + OV + NR - L - OV, K, BSH), dtype=np.float32)
        pad[OV: OV + T] = np.exp(emits[:, sl, :]).transpose(0, 2, 1)
        idx = np.arange(NR)[:, None] + np.arange(C)[None, :] * L  # [NR, C]
        earr = pad[idx]                        # [NR, C, K, BSH]
        earr = earr.transpose(0, 2, 1, 3).reshape(NR, K, FW)
        earr = np.concatenate(
            [earr, np.ones((NR, 1, FW), np.float32)], axis=1
        ).astype(BF)                           # row 64 = tt passthrough ones

        tau = tau_all[sl]
        cb = tau // L
        ib = tau - cb * L + OV
        in_maps.append(
            {"earr": earr, "expal": expal, "transitions": transitions}
        )
        aux.append((ib, cb))
    return in_maps, aux


def _assemble(results, aux):
    """Host-side final assembly: ln, chunk-scale stitch, and global sum."""
    total = np.float64(0.0)
    for res, (ib, cb) in zip(results, aux):
        csum = np.asarray(res["csum_out"], dtype=np.float64).reshape(NR, C, BSH)
        chist = np.asarray(res["chist_out"], dtype=np.float64).reshape(
            NWC, C, BSH
        )
        b = np.arange(BSH)
        cs_sel = csum[ib + 1, cb, b]
        ch_sel = chist[ib // W, cb, b]
        # chunk 0's exact re-anchor books -DELTA*OV into c_rows at round OV;
        # window-0 states (no fold yet) read c_hist[0]=0, so patch it here
        ch_sel = np.where((cb == 0) & (ib // W == 0), -DELTA * OV, ch_sel)
        csR, csOV, chR = csum[NR - 1], csum[OV], chist[NWC - 1]
        d = (np.log(csR[:-1]) + chR[:-1] + DELTA * (NR - 2)) - (
            np.log(csOV[1:]) + DELTA * (OV - 1)
        )
        lam = np.concatenate(
            [np.zeros((1, BSH)), np.cumsum(d, axis=0)], axis=0
        )  # [C, BSH]
        r = np.log(cs_sel) + ch_sel + DELTA * ib + lam[cb, b]
        total += r.sum()
    return np.float32(total)


def kernel(emits, mask, transitions, alpha_0):
    nc = _get_nc()
    in_maps, aux = _prep(
        {"emits": emits, "mask": mask, "transitions": transitions,
         "alpha_0": alpha_0}
    )
    res = run_bass_kernel_spmd(nc, in_maps, core_ids=list(range(NCORES)))
    return _assemble(res.results, aux)
